# revision 1
# baseline (speedup 1.0000x reference)
"""Trainium2 Bass kernel for CustomRationalLayer.

Math (B=256 batch, I=512 inputs, O=512 outputs):
    t = tanh(x * tanh_range)                                  (B, I)
    mole[b,o,i] = sum_{p=0..5} mc[o,i,p] t[b,i]^p
    deno[b,o,i] = sum_{q=1..4} dc[o,i,q-1] t[b,i]^q
    out[b,o]    = sum_i mole / (1 + |deno * x[b,i]|)

Strategy: tensor-parallel over O (64 outputs per core -> the large coef
tensors are split 8-way).  Per core:
  - power rows [1, t, t^2..t^5] and u_q = t^q * x computed once in
    (i-partition, b-free) layout (float32r so the PE runs at 1 cyc/col),
    then round-tripped through DRAM so a single strided DMA per phase can
    reload them in the [12/8 rows, pairs, B] matmul-rhs layout (DRAM APs
    have no partition-iteration-order constraint; SBUF->SBUF gathers would
    also exceed the 1-sync-wait DMA limit).
  - i is processed as 256 pairs j = (i, i+256), two pairs per PSUM bank.
    Per pair, one K=12 float32r matmul against block-diagonal host-packed
    weights produces the full numerator (const term included via the ones
    row) for both i's stacked on 128 partitions; a K=8 matmul produces
    deno*x.  Elementwise (on [128, 512] double-pair tiles): ACT abs,
    GPSIMD +1, DVE fast reciprocal, DVE ratio = pm * rcp (written as
    float32r), then one [I64;I64] identity matmul accumulates the i-sum
    into PSUM across all pairs.
Output per core is (64 o, 256 b); host transposes and concatenates.
"""

import numpy as np

import concourse.bass as bass
import concourse.tile as tile
from concourse import bacc, mybir
from concourse.bass_utils import run_bass_kernel_spmd

B = 256
I = 512
O = 512
NC = 8
OSH = O // NC          # outputs per core
NJ = I // 2            # i-pairs per core
PHJ = 16               # pairs per W-staging phase
NPH = NJ // PHJ
F32 = mybir.dt.float32
F32R = mybir.dt.float32r
ALU = mybir.AluOpType
AF = mybir.ActivationFunctionType

_CACHE = {}


def _act_reciprocal(nc, out, in_, bias):
    """ACT Reciprocal via raw InstActivation (the bass wrapper bans it; the
    measured accuracy of reciprocal(x+bias) on TRN2 is ~1.2e-5 max rel err,
    well inside this kernel's float32r noise floor)."""
    eng = nc.scalar
    ins = [eng.lower_ap(in_)]
    for val in (float(bias), 1.0, 0.0):  # bias, scale, alpha
        ins.append(mybir.ImmediateValue(dtype=mybir.dt.float32, value=val))
    return eng.add_instruction(mybir.InstActivation(
        name=nc.get_next_instruction_name(),
        func=AF.Reciprocal,
        ins=ins,
        outs=[eng.lower_ap(out)],
    ))


def _build_bass():
    nc = bacc.Bacc("TRN2", target_bir_lowering=False, debug=False, num_devices=NC)

    XT = nc.dram_tensor("xt", [I, B], F32, kind="ExternalInput").ap()
    TRB = nc.dram_tensor("trb", [128, 1], F32, kind="ExternalInput").ap()
    WM = nc.dram_tensor("wm", [12, NJ, 128], F32R, kind="ExternalInput").ap()
    WD = nc.dram_tensor("wd", [8, NJ, 128], F32R, kind="ExternalInput").ap()
    ID2 = nc.dram_tensor("id2", [128, OSH], F32R, kind="ExternalInput").ap()
    OUT = nc.dram_tensor("out_y", [OSH, B], F32, kind="ExternalOutput").ap()

    with tile.TileContext(nc) as tc:
        with (
            tc.tile_pool(name="consts", bufs=1) as consts,
            tc.tile_pool(name="powers", bufs=1) as powers,
            tc.tile_pool(name="dramp", bufs=1, space="DRAM") as dramp,
            tc.tile_pool(name="v2p", bufs=2) as v2p,
            tc.tile_pool(name="u2p", bufs=2) as u2p,
            tc.tile_pool(name="wmp", bufs=2) as wmp,
            tc.tile_pool(name="wdp", bufs=2) as wdp,
            tc.tile_pool(name="work", bufs=3) as work,
            tc.tile_pool(name="works", bufs=4) as works,
            tc.tile_pool(name="outp", bufs=1) as outp,
            tc.tile_pool(name="pmp", bufs=3, space="PSUM") as pmp,
            tc.tile_pool(name="pdp", bufs=2, space="PSUM") as pdp,
            tc.tile_pool(name="accp", bufs=1, space="PSUM") as accp,
        ):
            id2_s = consts.tile([128, OSH], F32R)
            nc.sync.dma_start(out=id2_s, in_=ID2)
            trb_s = consts.tile([128, 1], F32)
            nc.sync.dma_start(out=trb_s, in_=TRB)

            # x (raw and tanh_range-prescaled) in (i-partition, par, c1, b)
            # layout: i = 256*par + 128*c1 + p  (chunk = 2*par + c1)
            X = powers.tile([128, 2, 2, B], F32)
            nc.sync.dma_start(
                out=X, in_=XT.rearrange("(par c1 p) b -> p par c1 b", par=2, c1=2)
            )
            ones_f = powers.tile([128, 2, 2, B], F32)
            nc.vector.memset(ones_f, 1.0)

            # TT[:, r] = t^r (row 0 = ones, carries the constant coef term),
            # UU[:, q] = t^(q+1) * x.  float32r (PE full-rate); written by
            # DVE only so downstream DMAs depend on a single engine proc.
            TT = powers.tile([128, 6, 2, 2, B], F32R)
            UU = powers.tile([128, 4, 2, 2, B], F32R)
            nc.scalar.activation(TT[:, 1], X, AF.Tanh, scale=trb_s[:, 0:1])
            nc.vector.tensor_copy(TT[:, 0], ones_f)
            nc.vector.tensor_mul(TT[:, 2], TT[:, 1], TT[:, 1])
            nc.vector.tensor_mul(TT[:, 3], TT[:, 2], TT[:, 1])
            nc.vector.tensor_mul(TT[:, 4], TT[:, 2], TT[:, 2])
            nc.vector.tensor_mul(TT[:, 5], TT[:, 4], TT[:, 1])
            for q in range(4):
                nc.vector.tensor_mul(UU[:, q], TT[:, q + 1], X)

            # dump in 16-partition strips (separate DRAM tiles) so early
            # phases only wait on their own strip, not the whole dump
            TDs, UDs = [], []
            for st in range(4):
                td = dramp.tile([32, 6, 2, 2, B], F32R, tag=f"td{st}")
                nc.sync.dma_start(out=td, in_=TT[32 * st : 32 * (st + 1)])
                TDs.append(td.rearrange("p r par c b -> r par p c b"))
                ud = dramp.tile([32, 4, 2, 2, B], F32R, tag=f"ud{st}")
                nc.sync.dma_start(out=ud, in_=UU[32 * st : 32 * (st + 1)])
                UDs.append(ud.rearrange("p q par c b -> q par p c b"))

            # [64, 2, B]: the N=512 identity matmul leaves the two packed
            # pairs side by side; folded after the loop.
            acc = accp.tile([OSH, 2, B], F32)

            # identity matmuls are software-pipelined two groups behind the
            # mole/deno matmuls so the in-order PE stream never stalls
            # waiting for the DVE ratio of the current group
            pending = []
            n_ident = 0

            def flush_ident(limit):
                nonlocal n_ident
                while len(pending) > limit:
                    r0, r1 = pending.pop(0)
                    nc.tensor.matmul(
                        acc, id2_s, r0,
                        start=(n_ident == 0), stop=False,
                    )
                    n_ident += 1
                    nc.tensor.matmul(
                        acc, id2_s, r1,
                        start=False, stop=(n_ident == NJ // 2 - 1),
                    )
                    n_ident += 1

            for ph in range(NPH):
                p1 = (PHJ * ph) % 128
                c1 = (PHJ * ph) // 128
                wm_s = wmp.tile([12, PHJ, 128], F32R)
                nc.sync.dma_start(out=wm_s, in_=WM[:, PHJ * ph : PHJ * (ph + 1), :])
                wd_s = wdp.tile([8, PHJ, 128], F32R)
                nc.sync.dma_start(out=wd_s, in_=WD[:, PHJ * ph : PHJ * (ph + 1), :])
                # [12, PHJ, B]: row 2r+par = t^r of i = j + 256*par
                st, po = p1 // 32, p1 % 32
                with tc.high_priority(offset=400):
                    v2 = v2p.tile([12, PHJ, B], F32R)
                    nc.sync.dma_start(
                        out=v2, in_=TDs[st][:, :, po : po + PHJ, c1, :]
                    )
                    u2 = u2p.tile([8, PHJ, B], F32R)
                    nc.sync.dma_start(
                        out=u2, in_=UDs[st][:, :, po : po + PHJ, c1, :]
                    )

                for g4 in range(PHJ // 4):   # four pairs per elementwise group
                    pm_a = pmp.tile([128, 2, B], F32, tag="pm")
                    pm_b = pmp.tile([128, 2, B], F32, tag="pm")
                    pd4 = pdp.tile([128, 4, B], F32)
                    with tc.high_priority(offset=80):
                        for k in range(4):
                            jl = 4 * g4 + k
                            nc.tensor.matmul(
                                pd4[:, k], wd_s[:, jl, :], u2[:, jl, :],
                                start=True, stop=True,
                            )
                        for k in range(4):
                            jl = 4 * g4 + k
                            nc.tensor.matmul(
                                (pm_a if k < 2 else pm_b)[:, k % 2],
                                wm_s[:, jl, :], v2[:, jl, :],
                                start=True, stop=True,
                            )
                    gidx = (PHJ // 4) * ph + g4
                    z4 = work.tile([128, 4, B], F32, tag="z")
                    nc.scalar.activation(z4, pd4, AF.Abs)
                    rcp4 = work.tile([128, 4, B], F32, tag="rcp")
                    if False:
                        # DVE path: 1/(1+z) via +1 (gpsimd) + fast recip
                        a4 = work.tile([128, 4, B], F32, tag="a")
                        nc.gpsimd.tensor_scalar(a4, z4, 1.0, None, ALU.add)
                        nc.vector.reciprocal_approx_fast(out=rcp4, in_=a4)
                    else:
                        # ACT path: reciprocal(z + 1); abs and reciprocal share
                        # one activation table set -> no table reloads
                        _act_reciprocal(nc, rcp4, z4, 1.0)
                    ratio0 = works.tile([128, 2, B], F32R, tag="ratio0")
                    nc.vector.tensor_mul(ratio0, pm_a, rcp4[:, 0:2])
                    ratio1 = works.tile([128, 2, B], F32R, tag="ratio1")
                    nc.vector.tensor_mul(ratio1, pm_b, rcp4[:, 2:4])
                    pending.append((ratio0, ratio1))
                    flush_ident(2)

            flush_ident(0)

            acc_s = outp.tile([OSH, 2, B], F32)
            nc.scalar.copy(acc_s, acc)
            out_s = outp.tile([OSH, B], F32)
            nc.vector.tensor_add(out_s, acc_s[:, 0], acc_s[:, 1])
            nc.sync.dma_start(out=OUT, in_=out_s)

    nc.compile()
    return nc


def _prep_inputs(x, tanh_range, mole_coef, deno_coef):
    """Host-side prepack -> list of per-core input maps.

    W row order for the pair j=(i, i+256): row 2r+par = coef of power r
    for i + 256*par; columns 0:64 hold par=0 outputs, 64:128 par=1 outputs.
    Row pair 0/1 (power 0, the ones row) carries the constant coef mc0.
    """
    xt = np.ascontiguousarray(x.T.astype(np.float32))
    trb = np.full((128, 1), np.float32(tanh_range), dtype=np.float32)
    id2 = np.concatenate([np.eye(OSH), np.eye(OSH)], axis=0).astype(np.float32)
    in_maps = []
    for c in range(NC):
        o0 = OSH * c
        mc = mole_coef[o0 : o0 + OSH]  # (64, 512, 6)
        dc = deno_coef[o0 : o0 + OSH]  # (64, 512, 4)
        wm = np.zeros((12, NJ, 128), dtype=np.float32)
        wd = np.zeros((8, NJ, 128), dtype=np.float32)
        for r in range(6):
            wm[2 * r, :, 0:OSH] = mc[:, 0:NJ, r].T
            wm[2 * r + 1, :, OSH:128] = mc[:, NJ:I, r].T
        for r in range(4):
            wd[2 * r, :, 0:OSH] = dc[:, 0:NJ, r].T
            wd[2 * r + 1, :, OSH:128] = dc[:, NJ:I, r].T
        in_maps.append(
            {
                "xt": xt,
                "trb": trb,
                "wm": wm,
                "wd": wd,
                "id2": id2,
            }
        )
    return in_maps


def kernel(x, tanh_range, mole_coef, deno_coef):
    x = np.asarray(x, dtype=np.float32)
    mole_coef = np.asarray(mole_coef, dtype=np.float32)
    deno_coef = np.asarray(deno_coef, dtype=np.float32)
    if "nc" not in _CACHE:
        _CACHE["nc"] = _build_bass()
    nc = _CACHE["nc"]
    in_maps = _prep_inputs(x, tanh_range, mole_coef, deno_coef)
    res = run_bass_kernel_spmd(nc, in_maps, list(range(NC)))
    out = np.empty((B, O), dtype=np.float32)
    for c in range(NC):
        out[:, OSH * c : OSH * (c + 1)] = res.results[c]["out_y"].T
    return out



# revision 4
# speedup vs baseline: 1.2944x; 1.2944x over previous
"""Trainium2 Bass kernel for CustomRationalLayer (v2, bf16 + fused DVE).

Math (B=256 batch, I=512 inputs, O=512 outputs):
    t = tanh(x * tanh_range)                                  (B, I)
    mole[b,o,i] = sum_{p=0..5} mc[o,i,p] t[b,i]^p
    deno[b,o,i] = sum_{q=1..4} dc[o,i,q-1] t[b,i]^q
    out[b,o]    = sum_i mole / (1 + |deno * x[b,i]|)

Strategy: tensor-parallel over O (64 outputs per core).  Per core:
  - power rows [1, t..t^5, t^q*x] computed once in bf16 in an
    (i-partition, b-free) layout, round-tripped through DRAM so a strided
    DMA per phase reloads them in the [20 rows, pairs, B] matmul-rhs
    layout (rows 0-11 mole powers, 12-19 deno u's; row 2r+par = i-half).
  - i processed as 256 pairs j = (i, i+256), 4 pairs per PSUM-tile group.
    Per pair one K=8 bf16 matmul produces deno*x and one K=12 bf16 matmul
    the numerator, into [128, 1024] 2-bank PSUM tiles (pair k at columns
    256k).  ACT abs moves |deno*x| to SBUF f32; a custom fused DVE op
    (seed + 1 Newton step, ~0.2% rel err) computes
       s = mole * recip(1 + |deno*x|)
    in ONE DVE pass (bf16 out), and bf16 identity matmuls accumulate the
    i-sum in PSUM (software-pipelined two groups behind).
Output per core is (64 o, 256 b); host transposes and concatenates.
"""

import numpy as np
import ml_dtypes

import concourse.bass as bass
import concourse.tile as tile
from concourse import bacc, mybir
from concourse import dve_ops as _dve_ops
from concourse.bass_utils import run_bass_kernel_spmd
from concourse.dve_ops import DveOp
from concourse.dve_spec import AluOp, Bin, Spec, Src0, Src1, C0, C1, C2, lower, _has_src1
from concourse.dve_uop import DveOpSpec

B = 256
I = 512
O = 512
NC = 8
OSH = O // NC          # outputs per core
NJ = I // 2            # i-pairs per core
PHJ = 16               # pairs per W-staging phase
NPH = NJ // PHJ
F32 = mybir.dt.float32
BF16 = mybir.dt.bfloat16
AF = mybir.ActivationFunctionType
NPBF16 = ml_dtypes.bfloat16

# 1-Newton-step reciprocal constants (host-tuned: max rel err 2.1e-3 on [1, 300])
C_ADD = 1.0
C_SEED = -0.235
C_NR = 2.0015833333333335

_CACHE = {}


def _fused_recip_mul_op():
    """Custom DVE op: out = Src1 * y1,  y1 ~= 1 / (Src0 + c0).

    Seed y0 = bitwise_not(x) * c1 (exponent-flip trick), one Newton step
    y1 = y0 * (c2 - x*y0).  Registered into dve_ops.OPS so table-gen and
    CoreSim pick it up."""
    name = "RECIP1P_MUL_K45"
    for o in _dve_ops.OPS:
        if o.name == name:
            return o
    _x = Src0 + C0
    _nx = Bin(AluOp.BITWISE_NOT, _x, _x)
    _y0 = _nx * C1
    body = (_y0 * (C2 - _x * _y0)) * Src1

    def _ref(in0, in1, c0, c1, c2):
        x = (in0.astype(np.float32) + np.float32(c0)).astype(np.float32)
        nx = (~x.view(np.int32)).view(np.float32)
        y0 = nx * np.float32(c1)
        y1 = (y0 * (np.float32(c2) - x * y0)).astype(np.float32)
        return (y1 * in1.astype(np.float32)).astype(np.float32)

    spec = Spec(body=body, reference=_ref)
    row = _dve_ops._CUSTOM_DVE_ROW_BASE + len(_dve_ops.OPS)
    uops = lower(spec, ver="v3")
    sha = DveOpSpec(name=name, opcode=row, uops=uops, rd1_en=_has_src1(spec)).sha("v3")
    op = DveOp(name, spec, subdim=False, uops_sha={"v3": sha})
    _dve_ops.OPS.append(op)
    _dve_ops.CUSTOM_DVE_SPECS[name] = spec
    _dve_ops._SUB_OPCODE_FOR_NAME[name] = row
    return op


def _build_bass():
    fused = _fused_recip_mul_op()
    nc = bacc.Bacc("TRN2", target_bir_lowering=False, debug=False, num_devices=NC)

    XT = nc.dram_tensor("xt", [I, B], F32, kind="ExternalInput").ap()
    TRB = nc.dram_tensor("trb", [128, 1], F32, kind="ExternalInput").ap()
    WMD = nc.dram_tensor("wmd", [20, NJ, 128], BF16, kind="ExternalInput").ap()
    ID2 = nc.dram_tensor("id2", [128, OSH], BF16, kind="ExternalInput").ap()
    OUT = nc.dram_tensor("out_y", [OSH, B], F32, kind="ExternalOutput").ap()

    with tile.TileContext(nc) as tc:
        with (
            tc.tile_pool(name="consts", bufs=1) as consts,
            tc.tile_pool(name="powers", bufs=1) as powers,
            tc.tile_pool(name="dramp", bufs=1, space="DRAM") as dramp,
            tc.tile_pool(name="vup", bufs=2) as vup,
            tc.tile_pool(name="wmdp", bufs=2) as wmdp,
            tc.tile_pool(name="zp", bufs=2) as zp,
            tc.tile_pool(name="sp", bufs=4) as sp,
            tc.tile_pool(name="outp", bufs=1) as outp,
            tc.tile_pool(name="pmp", bufs=2, space="PSUM") as pmp,
            tc.tile_pool(name="pdp", bufs=1, space="PSUM") as pdp,
            tc.tile_pool(name="accp", bufs=1, space="PSUM") as accp,
        ):
            id2_s = consts.tile([128, OSH], BF16)
            nc.sync.dma_start(out=id2_s, in_=ID2)
            trb_s = consts.tile([128, 1], F32)
            nc.sync.dma_start(out=trb_s, in_=TRB)

            # x in (i-partition, par, c1, b) layout: i = 256*par + 128*c1 + p
            X = powers.tile([128, 2, 2, B], F32)
            nc.sync.dma_start(
                out=X, in_=XT.rearrange("(par c1 p) b -> p par c1 b", par=2, c1=2)
            )
            Xb = powers.tile([128, 2, 2, B], BF16)
            nc.vector.tensor_copy(Xb, X)

            # PW[:, rt] for rt 0..5: t^rt (row 0 = ones, carries const coef);
            # rt 6..9: u_q = t^q * x.  All bf16.
            PW = powers.tile([128, 10, 2, 2, B], BF16)
            nc.vector.memset(PW[:, 0], 1.0)
            nc.scalar.activation(PW[:, 1], X, AF.Tanh, scale=trb_s[:, 0:1])
            nc.vector.tensor_mul(PW[:, 2], PW[:, 1], PW[:, 1])
            nc.vector.tensor_mul(PW[:, 3], PW[:, 2], PW[:, 1])
            nc.vector.tensor_mul(PW[:, 4], PW[:, 2], PW[:, 2])
            nc.vector.tensor_mul(PW[:, 5], PW[:, 4], PW[:, 1])
            for q in range(4):
                nc.vector.tensor_mul(PW[:, 6 + q], PW[:, 1 + q], Xb)

            # dump in 32-partition strips (contiguous per partition) so each
            # phase's reload only waits on its own strip
            PWDs = []
            for st in range(4):
                pwd = dramp.tile([32, 10, 2, 2, B], BF16, tag=f"pwd{st}")
                nc.sync.dma_start(out=pwd, in_=PW[32 * st : 32 * (st + 1)])
                PWDs.append(pwd.rearrange("p rt par c b -> (rt par) p c b"))

            # acc[o, h, b]: h = pair parity within s4 halves; folded at end
            acc = accp.tile([OSH, 2, B], F32)

            pending = []
            n_ident = 0

            def flush_ident(limit):
                nonlocal n_ident
                while len(pending) > limit:
                    s4 = pending.pop(0)
                    nc.tensor.matmul(
                        acc, id2_s, s4[:, 0:512],
                        start=(n_ident == 0), stop=False,
                    )
                    n_ident += 1
                    nc.tensor.matmul(
                        acc, id2_s, s4[:, 512:1024],
                        start=False, stop=(n_ident == NJ // 2 - 1),
                    )
                    n_ident += 1

            for ph in range(NPH):
                j0 = PHJ * ph
                c1 = j0 // 128
                po = (j0 % 128) % 32
                st = (j0 % 128) // 32
                wm_s = wmdp.tile([12, PHJ, 128], BF16, tag="wm")
                nc.sync.dma_start(out=wm_s, in_=WMD[0:12, j0 : j0 + PHJ, :])
                wd_s = wmdp.tile([8, PHJ, 128], BF16, tag="wd")
                nc.sync.dma_start(out=wd_s, in_=WMD[12:20, j0 : j0 + PHJ, :])
                with tc.high_priority(offset=400):
                    v2 = vup.tile([12, PHJ, B], BF16, tag="v2")
                    nc.sync.dma_start(
                        out=v2, in_=PWDs[st][0:12, po : po + PHJ, c1, :]
                    )
                    u2 = vup.tile([8, PHJ, B], BF16, tag="u2")
                    nc.sync.dma_start(
                        out=u2, in_=PWDs[st][12:20, po : po + PHJ, c1, :]
                    )

                for g4 in range(PHJ // 4):   # four pairs per elementwise group
                    pd = pdp.tile([128, 4 * B], F32)
                    pm = pmp.tile([128, 4 * B], F32)
                    with tc.high_priority(offset=80):
                        for k in range(4):
                            jl = 4 * g4 + k
                            nc.tensor.matmul(
                                pd[:, B * k : B * (k + 1)],
                                wd_s[:, jl, :], u2[:, jl, :],
                                start=True, stop=True,
                            )
                        for k in range(4):
                            jl = 4 * g4 + k
                            nc.tensor.matmul(
                                pm[:, B * k : B * (k + 1)],
                                wm_s[:, jl, :], v2[:, jl, :],
                                start=True, stop=True,
                            )
                    z4 = zp.tile([128, 4 * B], F32, tag="z")
                    nc.scalar.activation(z4, pd, AF.Abs)
                    s4 = sp.tile([128, 4 * B], BF16, tag="s")
                    nc.vector._custom_dve(
                        fused, out=s4, in0=z4, in1=pm,
                        s0=C_ADD, s1=C_SEED, imm2=C_NR,
                    )
                    pending.append(s4)
                    flush_ident(2)

            flush_ident(0)

            acc_s = outp.tile([OSH, 2, B], F32)
            nc.scalar.copy(acc_s, acc)
            out_s = outp.tile([OSH, B], F32)
            nc.vector.tensor_add(out_s, acc_s[:, 0], acc_s[:, 1])
            nc.sync.dma_start(out=OUT, in_=out_s)

    nc.compile()
    return nc


def _prep_inputs(x, tanh_range, mole_coef, deno_coef):
    """Host-side prepack -> list of per-core input maps.

    wmd row order for pair j=(i, i+256): row 2r+par = mole coef of power r,
    row 12+2q+par = deno coef of power q+1, for i + 256*par; columns 0:64
    hold par=0 outputs, 64:128 par=1.  Row pair 0/1 (ones row) carries the
    constant mole coef."""
    xt = np.ascontiguousarray(x.T.astype(np.float32))
    trb = np.full((128, 1), np.float32(tanh_range), dtype=np.float32)
    id2 = np.concatenate([np.eye(OSH), np.eye(OSH)], axis=0).astype(NPBF16)
    in_maps = []
    for c in range(NC):
        o0 = OSH * c
        mc = mole_coef[o0 : o0 + OSH]  # (64, 512, 6)
        dc = deno_coef[o0 : o0 + OSH]  # (64, 512, 4)
        wmd = np.zeros((20, NJ, 128), dtype=np.float32)
        for r in range(6):
            wmd[2 * r, :, 0:OSH] = mc[:, 0:NJ, r].T
            wmd[2 * r + 1, :, OSH:128] = mc[:, NJ:I, r].T
        for r in range(4):
            wmd[12 + 2 * r, :, 0:OSH] = dc[:, 0:NJ, r].T
            wmd[12 + 2 * r + 1, :, OSH:128] = dc[:, NJ:I, r].T
        in_maps.append(
            {
                "xt": xt,
                "trb": trb,
                "wmd": wmd.astype(NPBF16),
                "id2": id2,
            }
        )
    return in_maps


def kernel(x, tanh_range, mole_coef, deno_coef):
    x = np.asarray(x, dtype=np.float32)
    mole_coef = np.asarray(mole_coef, dtype=np.float32)
    deno_coef = np.asarray(deno_coef, dtype=np.float32)
    if "nc" not in _CACHE:
        _CACHE["nc"] = _build_bass()
    nc = _CACHE["nc"]
    in_maps = _prep_inputs(x, tanh_range, mole_coef, deno_coef)
    res = run_bass_kernel_spmd(nc, in_maps, list(range(NC)))
    out = np.empty((B, O), dtype=np.float32)
    for c in range(NC):
        out[:, OSH * c : OSH * (c + 1)] = res.results[c]["out_y"].T
    return out
